# revision 29
# baseline (speedup 1.0000x reference)
"""GAT (dense masked softmax attention) Bass kernel for 8 Trainium2 NeuronCores.

Row-parallel sharding: core c owns output rows [c*NB, (c+1)*NB). Each core
computes the full h = x @ W.T (replicated), then its row-block of the masked
attention softmax against all N nodes, in transposed layout (j on partitions,
own-rows on free dim), accumulating z @ [h | 1] in PSUM so the softmax
denominator falls out of the same matmuls.

Host prep (sharding only): x -> x.T, adjacency row-block -> additive mask
(0 at edges / -1e4 elsewhere) transposed to [N, NB], weight packing
(Wcat = [W.T | W.T@a2], b1 = W.T@a1 replicated across 128 columns).
"""

import contextlib
import ctypes
import sys
import types

import numpy as np
import ml_dtypes

import concourse.bacc as bacc
import concourse.mybir as mybir
import concourse.tile as tile

P = 128
NEG_MASK = -1.0e4  # additive mask; prelu scales it by alpha=0.01 -> exp(~-100) ~ 0


def _install_ntff_hook():
    """Register the axon NTFF profile hook so run_bass_kernel_spmd(trace=True)
    can capture neuron-profile data (antenv.axon_hooks is absent here)."""
    if "antenv.axon_hooks" in sys.modules:
        return
    try:
        lib = ctypes.CDLL("/opt/axon/libaxon_pjrt.so")
        if not hasattr(lib, "axon_start_nrt_profile"):
            return
    except OSError:
        return
    lib.axon_start_nrt_profile.argtypes = [ctypes.POINTER(ctypes.c_int64), ctypes.c_size_t]
    lib.axon_start_nrt_profile.restype = ctypes.c_int64
    lib.axon_stop_nrt_profile.argtypes = [ctypes.c_char_p]
    lib.axon_stop_nrt_profile.restype = ctypes.c_int64

    @contextlib.contextmanager
    def _hook(output_dir, device_ids):
        import jax

        jax.devices()
        if device_ids:
            ids = (ctypes.c_int64 * len(device_ids))(*device_ids)
            rc = lib.axon_start_nrt_profile(ids, len(device_ids))
        else:
            rc = lib.axon_start_nrt_profile(None, 0)
        if rc != 0:
            raise RuntimeError(f"axon_start_nrt_profile rc={rc}")
        try:
            yield
        finally:
            n = lib.axon_stop_nrt_profile(str(output_dir).encode())
            print(f"ntff profile: {n} file(s) in {output_dir}", file=sys.stderr)

    mod = types.ModuleType("antenv.axon_hooks")
    mod.get_axon_ntff_profile_hook = lambda: _hook
    mod.set_axon_ntff_profile_hook = lambda h: None
    sys.modules["antenv.axon_hooks"] = mod


class GatConfig:
    def __init__(self, n=8192, d=512, h=256, c=16, n_cores=8, s_f32=True):
        assert n % (n_cores * P) == 0 and d % P == 0 and h % P == 0
        self.n, self.d, self.h, self.c, self.n_cores = n, d, h, c, n_cores
        self.nb = n // n_cores          # own rows per core
        self.nch = n // P               # j-chunks (also m-tiles of h)
        self.ndc = d // P               # feature chunks
        self.nit = self.nb // P         # own i-tiles
        self.dt_x = mybir.dt.bfloat16   # x / weights path
        self.dt_z = mybir.dt.bfloat16   # post-exp z and h matmul operands
        self.dt_s = mybir.dt.float32 if s_f32 else mybir.dt.bfloat16  # pre-exp path

    def key(self):
        return (self.n, self.d, self.h, self.c, self.n_cores, self.dt_s)


def build_gat(cfg: GatConfig):
    """Build + compile the SPMD Bass program (identical on all cores)."""
    nc = bacc.Bacc("TRN2", target_bir_lowering=False, debug=False,
                   enable_asserts=False, num_devices=cfg.n_cores)
    N, D, H, C = cfg.n, cfg.d, cfg.h, cfg.c
    NB, NCH, NDC, NIT = cfg.nb, cfg.nch, cfg.ndc, cfg.nit
    f32 = mybir.dt.float32
    bf16 = mybir.dt.bfloat16

    xT = nc.dram_tensor("xT", [D, N], cfg.dt_x, kind="ExternalInput").ap()
    xTown = nc.dram_tensor("xTown", [D, NB], cfg.dt_x, kind="ExternalInput").ap()
    wcat = nc.dram_tensor("wcat", [D, H + 1], cfg.dt_x, kind="ExternalInput").ap()
    b1rep = nc.dram_tensor("b1rep", [D, P], cfg.dt_x, kind="ExternalInput").ap()
    maskT = nc.dram_tensor("maskT", [N, NB], bf16, kind="ExternalInput").ap()
    fcwT = nc.dram_tensor("fcwT", [H, C], f32, kind="ExternalInput").ap()
    fcb = nc.dram_tensor("fcb", [C, 1], f32, kind="ExternalInput").ap()
    logitsT = nc.dram_tensor("logitsT", [C, NB], f32, kind="ExternalOutput").ap()

    AF = mybir.ActivationFunctionType
    OP = mybir.AluOpType

    with tile.TileContext(nc) as tc:
        with (
            tc.tile_pool(name="persist", bufs=1) as pp,
            tc.tile_pool(name="mwork", bufs=6) as mwp,
            tc.tile_pool(name="swork", bufs=3) as swp,
            tc.tile_pool(name="zwork", bufs=3) as zwp,
            tc.tile_pool(name="tail", bufs=2) as tp,
        ):
            # ---------------- resident inputs ----------------
            xo_sb = []
            for dd in range(NDC):
                t = pp.tile([P, NB], cfg.dt_x, tag=f"xo{dd}")
                nc.sync.dma_start(t[:], xTown[dd * P:(dd + 1) * P, :])
                xo_sb.append(t)
            wcat_sb = []
            for dd in range(NDC):
                t = pp.tile([P, H + 1], cfg.dt_x, tag=f"wc{dd}")
                nc.sync.dma_start(t[:], wcat[dd * P:(dd + 1) * P, :])
                wcat_sb.append(t)
            b1_sb = []
            for dd in range(NDC):
                t = pp.tile([P, P], cfg.dt_x, tag=f"b1{dd}")
                nc.sync.dma_start(t[:], b1rep[dd * P:(dd + 1) * P, :])
                b1_sb.append(t)
            fcw_sb = []
            for hh in range(H // P):
                t = pp.tile([P, C], f32, tag=f"fcw{hh}")
                nc.sync.dma_start(t[:], fcwT[hh * P:(hh + 1) * P, :])
                fcw_sb.append(t)
            fcb_sb = pp.tile([C, 1], f32, tag="fcb")
            nc.sync.dma_start(fcb_sb[:], fcb[:])

            f1b_sb = pp.tile([P, NB], cfg.dt_s, tag="f1b")
            h_sb = [pp.tile([P, H], cfg.dt_z, tag=f"h{m}", name=f"h{m}")
                    for m in range(NCH)]
            onecol = pp.tile([P, 1], cfg.dt_z, tag="onecol")
            nc.gpsimd.memset(onecol[:], 1.0)
            onerow = pp.tile([1, P], f32, tag="onerow")
            nc.gpsimd.memset(onerow[:], 1.0)

            MB = NIT                     # m-tiles per xT column block
            NH = H // P                  # stationary h halves
            nq = (NB + 511) // 512       # 512-wide column groups of NB
            xtb = {}

            # Transposed accumulators accT[half][q] [P, 512] (4 banks) +
            # denominator rows (nq banks) + h-pipeline psum (2 banks) = 8.
            # h is the STATIONARY matmul operand (2 LDWs/chunk, hidden under
            # N=512 streams); z feeds straight through as the moving operand.
            with tc.tile_pool(name="acc", bufs=1, space="PSUM") as accp:
                accT = [[accp.tile([P, min(512, NB - q * 512)], f32,
                                   tag=f"accT{hh}_{q}", name=f"accT{hh}_{q}")
                         for q in range(nq)] for hh in range(NH)]
                dn = [accp.tile([1, min(512, NB - q * 512)], f32, tag=f"dn{q}",
                                name=f"dn{q}")
                      for q in range(nq)]

                with tc.tile_pool(name="ps1", bufs=2, space="PSUM") as ps1:
                    # f1b[p, i] = sum_d b1rep[d, p] * xTown[d, i] (same value
                    # on every partition p)
                    for q in range(nq):
                        w = min(512, NB - q * 512)
                        fps = accT[0][q][:, 0:w]  # transient psum reuse
                        for dd in range(NDC):
                            nc.tensor.matmul(fps, b1_sb[dd][:],
                                             xo_sb[dd][:, q * 512:q * 512 + w],
                                             start=(dd == 0), stop=(dd == NDC - 1))
                        nc.vector.tensor_copy(f1b_sb[:, q * 512:q * 512 + w],
                                              fps)

                    # Software-pipelined emission: the "producer" stage
                    # (xT/mask DMA, h matmuls, f2 copy, h cast, pre-exp u)
                    # runs one chunk AHEAD of the "consumer" stage (Prelu/Exp,
                    # mask-mul, accumulation matmuls). This keeps the tiny
                    # f2c copy that gates Prelu AHEAD of the z-mul/CAST in
                    # DVE program order, so ACT never idles between chunks.
                    state = {}

                    def produce(ch):
                        cb, mi = divmod(ch, MB)
                        if mi == 0:
                            for dd in range(NDC):
                                t = mwp.tile([P, MB * P], cfg.dt_x,
                                             tag=f"xtb{dd}", bufs=2,
                                             name=f"xtb{dd}_{cb}")
                                nc.sync.dma_start(
                                    t[:], xT[dd * P:(dd + 1) * P,
                                             cb * MB * P:(cb + 1) * MB * P])
                                xtb[dd, cb] = t
                        mk = mwp.tile([P, NB], bf16, tag="mask")
                        nc.sync.dma_start(mk[:], maskT[ch * P:(ch + 1) * P, :])

                        hps = ps1.tile([P, H + 1], f32, tag="hps")
                        for dd in range(NDC):
                            nc.tensor.matmul(hps[:],
                                             xtb[dd, cb][:, mi * P:(mi + 1) * P],
                                             wcat_sb[dd][:],
                                             start=(dd == 0), stop=(dd == NDC - 1))
                        dve_lrelu = (ch % 3 == 1) and NCH >= 32
                        u = f2c = None
                        if dve_lrelu:
                            # leaky-relu on DVE to relieve the scalar engine
                            sm = swp.tile([P, NB], cfg.dt_s, tag="sm", bufs=2)
                            nc.vector.tensor_scalar(out=sm[:], in0=f1b_sb[:],
                                                    scalar1=hps[:, H:H + 1],
                                                    scalar2=None, op0=OP.add)
                            p01 = swp.tile([P, NB], cfg.dt_s, tag="p01", bufs=2)
                            nc.vector.tensor_scalar_mul(p01[:], sm[:], 0.01)
                            u = swp.tile([P, NB], cfg.dt_s, tag="u")
                            nc.vector.tensor_tensor(out=u[:], in0=sm[:],
                                                    in1=p01[:], op=OP.max)
                        else:
                            f2c = tp.tile([P, 1], f32, tag="f2c", bufs=4)
                            nc.vector.tensor_copy(f2c[:], hps[:, H:H + 1])
                        nc.vector.tensor_copy(h_sb[ch][:], hps[:, 0:H])
                        state[ch] = (mk, u, f2c)

                    def consume(ch):
                        mk, u, f2c = state.pop(ch)
                        if u is None:
                            u = swp.tile([P, NB], cfg.dt_s, tag="u")
                            nc.scalar.activation(u[:], f1b_sb[:], AF.Prelu,
                                                 bias=f2c[:], scale=1.0,
                                                 alpha=0.01)
                        ez = swp.tile([P, NB], cfg.dt_s, tag="ez")
                        nc.scalar.activation(ez[:], u[:], AF.Exp)
                        z = zwp.tile([P, NB], cfg.dt_z, tag="z", bufs=4)
                        nc.vector.tensor_tensor(out=z[:], in0=ez[:],
                                                in1=mk[:], op=OP.mult)
                        for hh in range(NH):
                            for q in range(nq):
                                w = min(512, NB - q * 512)
                                nc.tensor.matmul(
                                    accT[hh][q][:],
                                    h_sb[ch][:, hh * P:(hh + 1) * P],
                                    z[:, q * 512:q * 512 + w],
                                    start=(ch == 0), stop=(ch == NCH - 1))
                        for q in range(nq):
                            w = min(512, NB - q * 512)
                            nc.tensor.matmul(dn[q][:], onecol[:],
                                             z[:, q * 512:q * 512 + w],
                                             start=(ch == 0),
                                             stop=(ch == NCH - 1))

                    for ch in range(NCH):
                        produce(ch)
                        if ch >= 1:
                            consume(ch - 1)
                    consume(NCH - 1)

                # ------------ tail A: normalize + ELU (transposed) ------------
                dnrow = pp.tile([1, NB], f32, tag="dnrow")
                for q in range(nq):
                    w = min(512, NB - q * 512)
                    nc.vector.tensor_copy(dnrow[0:1, q * 512:q * 512 + w], dn[q][:])
                recrow = pp.tile([1, NB], f32, tag="recrow")
                nc.vector.reciprocal(recrow[:], dnrow[:])
                oeT = []
                with tc.tile_pool(name="psR", bufs=2, space="PSUM") as psR:
                    rb_sb = []
                    for q in range(nq):
                        w = min(512, NB - q * 512)
                        rb = psR.tile([P, w], f32, tag="rb")
                        nc.tensor.matmul(rb[:], onerow[:],
                                         recrow[0:1, q * 512:q * 512 + w],
                                         start=True, stop=True)
                        rs = tp.tile([P, NB if False else 512], f32, tag="rs",
                                     bufs=2)
                        nc.vector.tensor_copy(rs[:, 0:w], rb[:])
                        rb_sb.append(rs)
                    for hh in range(NH):
                        row = []
                        for q in range(nq):
                            w = min(512, NB - q * 512)
                            on = tp.tile([P, 512], f32, tag="on", bufs=2)
                            nc.vector.tensor_tensor(out=on[:, 0:w],
                                                    in0=accT[hh][q][:],
                                                    in1=rb_sb[q][:, 0:w],
                                                    op=OP.mult)
                            pos = tp.tile([P, 512], f32, tag="pos", bufs=2)
                            nc.vector.tensor_scalar(out=pos[:, 0:w],
                                                    in0=on[:, 0:w], scalar1=0.0,
                                                    scalar2=None, op0=OP.max)
                            ngm = tp.tile([P, 512], f32, tag="ngm", bufs=2)
                            nc.vector.tensor_scalar(out=ngm[:, 0:w],
                                                    in0=on[:, 0:w], scalar1=0.0,
                                                    scalar2=None, op0=OP.min)
                            ex = tp.tile([P, 512], f32, tag="ex", bufs=2)
                            nc.scalar.activation(ex[:, 0:w], ngm[:, 0:w], AF.Exp)
                            o = pp.tile([P, 512], f32, tag=f"oeT{hh}_{q}",
                                        name=f"oeT{hh}_{q}")
                            nc.vector.scalar_tensor_tensor(out=o[:, 0:w],
                                                           in0=ex[:, 0:w],
                                                           scalar=-1.0,
                                                           in1=pos[:, 0:w],
                                                           op0=OP.add,
                                                           op1=OP.add)
                            row.append(o)
                        oeT.append(row)

            # -------- tail B: logitsT = fc_w @ oeT + b (no transposes) --------
            logT = pp.tile([C, NB], f32, tag="logT")
            with tc.tile_pool(name="ps3", bufs=2, space="PSUM") as ps3:
                for q in range(nq):
                    w = min(512, NB - q * 512)
                    lps = ps3.tile([C, 512], f32, tag="lps")
                    for hh in range(NH):
                        nc.tensor.matmul(lps[:, 0:w], fcw_sb[hh][:],
                                         oeT[hh][q][:, 0:w],
                                         start=(hh == 0), stop=(hh == NH - 1))
                    nc.vector.tensor_scalar(out=logT[:, q * 512:q * 512 + w],
                                            in0=lps[:, 0:w], scalar1=fcb_sb[:],
                                            scalar2=None, op0=OP.add)
            nc.sync.dma_start(logitsT[:], logT[:])

    nc.compile()
    return nc


# ---------------------------------------------------------------------------
# Host-side prep + execution
# ---------------------------------------------------------------------------

_CACHE = {}


def _get_nc(cfg: GatConfig):
    k = cfg.key()
    if k not in _CACHE:
        _CACHE[k] = build_gat(cfg)
    return _CACHE[k]


def prep_inputs(cfg, x, edge_index, W, a1, a2, fc_w, fc_b):
    """Shard + pack host inputs -> list of per-core in_maps."""
    bf = ml_dtypes.bfloat16
    N, NB = cfg.n, cfg.nb
    x = np.asarray(x, np.float32)
    W = np.asarray(W, np.float32)
    xT = np.ascontiguousarray(x.T).astype(bf)                    # [D, N]
    b1 = (W.T @ np.asarray(a1, np.float32)).astype(np.float32)   # [D, 1]
    b2 = (W.T @ np.asarray(a2, np.float32)).astype(np.float32)
    wcat = np.concatenate([W.T, b2], axis=1).astype(bf)          # [D, H+1]
    b1rep = np.repeat(b1, P, axis=1).astype(bf)                  # [D, P]
    fcwT = np.ascontiguousarray(np.asarray(fc_w, np.float32).T)  # [H, C]
    fcb = np.asarray(fc_b, np.float32).reshape(-1, 1)            # [C, 1]

    src = np.asarray(edge_index[0])
    dst = np.asarray(edge_index[1])
    in_maps = []
    for c in range(cfg.n_cores):
        lo = c * NB
        maskT = np.zeros((N, NB), np.float32)
        sel = (src >= lo) & (src < lo + NB)
        maskT[dst[sel], src[sel] - lo] = 1.0
        diag = np.arange(NB)
        maskT[lo + diag, diag] = 1.0
        in_maps.append({
            "xT": xT,
            "xTown": np.ascontiguousarray(xT[:, lo:lo + NB]),
            "wcat": wcat,
            "b1rep": b1rep,
            "maskT": maskT.astype(bf),
            "fcwT": fcwT,
            "fcb": fcb,
        })
    return in_maps


def run(cfg, inputs, trace=False):
    """Compile (cached), run on the 8 cores, return (logits, BassKernelResults)."""
    _install_ntff_hook()
    from concourse.bass_utils import run_bass_kernel_spmd

    nc = _get_nc(cfg)
    in_maps = prep_inputs(cfg, **inputs)
    res = run_bass_kernel_spmd(nc, in_maps, core_ids=list(range(cfg.n_cores)),
                               trace=trace)
    logits = np.concatenate(
        [np.asarray(res.results[c]["logitsT"], np.float32).T
         for c in range(cfg.n_cores)], axis=0)
    return logits, res


def kernel(x, edge_index, W, a1, a2, fc_w, fc_b):
    cfg = GatConfig(n=x.shape[0], d=x.shape[1], h=W.shape[0], c=fc_w.shape[0])
    logits, _ = run(cfg, dict(x=x, edge_index=edge_index, W=W, a1=a1, a2=a2,
                              fc_w=fc_w, fc_b=fc_b))
    return logits
